# revision 19
# baseline (speedup 1.0000x reference)
"""Trainium2 Bass kernel: multi-head attention forward (B=2, S=2048, D=1024, H=16).

Sharding: 8 cores = data-parallel over batch (2) x tensor-parallel over heads
(4 head-groups of 4 heads).  Host side: inputs are pre-transposed / sliced /
fp8-split per core; the 4 partial outputs per batch are summed and the bias
added on the host (the "unshard").

Precision scheme (error budget ~1.2e-2 << 2e-2 gate):
  - x and Wq/Wk/Wv are hi/lo-split into e4m3 pairs on the host
    (t = hi + lo with hi = fp8(t), lo = fp8(t - hi); residual ~0.1%).
    Weights are pre-scaled by 32 so the lo parts stay in e4m3's normal
    range; the scale is compensated in the exp scale (/1024) and in
    Wo (/32).
  - q/k/v projections: fp8 DoubleRow matmuls, 3 terms
    (xh@Wh + xh@Wl + xl@Wh), pairing k-tiles in the two DR slots:
    12 instructions of N/2 cycles vs 8 of N for f32r (0.75x).
  - scores: one DoubleRow per sk-tile: stationary slots (kh, kl) so k is
    exact; moving q is a single e4m3 (slot-broadcast AP) -> 0.5x PE cost,
    the only lossy step (~1.1% of ctx sigma).
  - exp -> bf16 attention weights (ACT), processed in ski-pairs to halve
    the per-instruction overhead.
  - PV and output projection in bf16 (same PE cost as f32r; v and ctx are
    bf16, Wo is bf16 pre-scaled 1/32 on the host).
  - softmax denominators via a bf16 ones-column in v (row 64 of the PV
    psum); normalization: DVE copy + fast reciprocal + Pool broadcast +
    DVE multiply.

Scheduling: one pass over NJ=4 query blocks of W=512; each block emits its
own x DMAs, q/k/v projections, attention (per head, ski-pairs), softmax
normalization, and output projection, so DMA/PE/ACT/DVE/Pool overlap across
blocks.  Out-projection psum is staged to SBUF on the Pool engine (DMA
cannot read PSUM).
"""

import sys

sys.path.insert(0, "/opt/trn_rl_repo")

import numpy as np
import ml_dtypes

B, S, D = 2, 2048, 1024
H = 16
DH = 64
HL = 4  # heads per core
NCORES = 8

WSC = 32.0  # power-of-2 weight prescale for fp8 range
E4 = ml_dtypes.float8_e4m3
BF = ml_dtypes.bfloat16

_PROGRAM_CACHE = {}


def build_program(S=S, D=D, HL=HL, DH=DH):
    import concourse.tile as tile
    from concourse import bacc, mybir

    f32 = mybir.dt.float32
    f8 = mybir.dt.float8e4
    bf16 = mybir.dt.bfloat16
    A = mybir.ActivationFunctionType
    Alu = mybir.AluOpType
    DR = mybir.MatmulPerfMode.DoubleRow

    KD = D // 128         # contraction k-tiles for the projections (8)
    NP = KD // 2          # DoubleRow k-tile pairs (4)
    M = HL * DH           # per-core projected width (256)
    MQ = M // 128         # q/k partition planes (2)
    ST = S // 128         # 128-row s tiles (16)
    W = min(512, S)       # query-block width
    NJ = S // W           # query blocks
    TPB = W // 128        # sk tiles per query block (4)
    scale = (1.0 / float(np.sqrt(DH))) / (WSC * WSC)

    nc = bacc.Bacc("TRN2", target_bir_lowering=False, debug=False)
    xh = nc.dram_tensor("xh", (D, S), f8, kind="ExternalInput").ap()
    xl = nc.dram_tensor("xl", (D, S), f8, kind="ExternalInput").ap()
    wqh = nc.dram_tensor("wqh", (D, M), f8, kind="ExternalInput").ap()
    wql = nc.dram_tensor("wql", (D, M), f8, kind="ExternalInput").ap()
    wkh = nc.dram_tensor("wkh", (D, M), f8, kind="ExternalInput").ap()
    wkl = nc.dram_tensor("wkl", (D, M), f8, kind="ExternalInput").ap()
    wvh = nc.dram_tensor("wvh", (D, M), f8, kind="ExternalInput").ap()
    wvl = nc.dram_tensor("wvl", (D, M), f8, kind="ExternalInput").ap()
    wo = nc.dram_tensor("wo", (M, D), bf16, kind="ExternalInput").ap()
    out = nc.dram_tensor("out", (S, D), f32, kind="ExternalOutput").ap()

    xh_r = xh.rearrange("(k p) s -> p k s", p=128)
    xl_r = xl.rearrange("(k p) s -> p k s", p=128)
    w_r = {
        "wqh": wqh.rearrange("(k p) m -> p k m", p=128),
        "wql": wql.rearrange("(k p) m -> p k m", p=128),
        "wkh": wkh.rearrange("(k p) m -> p k m", p=128),
        "wkl": wkl.rearrange("(k p) m -> p k m", p=128),
        "wvh": wvh.rearrange("(k p) m -> p k m", p=128),
        "wvl": wvl.rearrange("(k p) m -> p k m", p=128),
    }

    with tile.TileContext(nc) as tc:
        with (
            tc.tile_pool(name="persist", bufs=1) as mp,
            tc.tile_pool(name="attn", bufs=4) as apool,
            tc.tile_pool(name="norm", bufs=2) as rpool,
            tc.tile_pool(name="ostage", bufs=3) as opool,
            tc.tile_pool(name="gps", bufs=2, space="PSUM") as gpool,
            tc.tile_pool(name="scps", bufs=2, space="PSUM") as spool,
            tc.tile_pool(name="ctxps", bufs=2, space="PSUM") as cpool,
        ):
            xh_sb = mp.tile([128, KD, S], f8, tag="xh")
            xl_sb = mp.tile([128, KD, S], f8, tag="xl")
            w_sb = {n: mp.tile([128, KD, M], f8, tag=n, name=n) for n in w_r}
            wo_sb = mp.tile([128, MQ, D], bf16, tag="wo")
            q8_sb = mp.tile([128, MQ, S], f8, tag="q8")
            k8_sb = mp.tile([128, MQ, 2, S], f8, tag="k8")
            v_sb = mp.tile([128, ST, HL * (DH + 1)], bf16, tag="v")
            ctx_sb = mp.tile([128, MQ, S], bf16, tag="ctx")

            # ones columns for the PV denominator trick
            nc.vector.memset(v_sb[:, :, DH::DH + 1], 1.0)

            # ------------- per-block emitters ---------------------------
            def emit_x_dmas(j, part="hl"):
                jsl = slice(j * W, (j + 1) * W)
                if "h" in part:
                    nc.sync.dma_start(xh_sb[:, :, jsl], xh_r[:, :, jsl])
                if "l" in part:
                    nc.sync.dma_start(xl_sb[:, :, jsl], xl_r[:, :, jsl])

            def emit_qk_group(j, m, dst):
                jsl = slice(j * W, (j + 1) * W)
                msl = slice(m * 128, (m + 1) * 128)
                ps = gpool.tile([128, W], f32, tag="ps", name="ps_qk")
                if dst == "q":
                    # q is re-quantized to a single e4m3 for the scores, so
                    # a 2-term (exact-W, single-fp8-x) projection suffices
                    n_mm = 2 * NP
                    i = 0
                    for p in range(NP):
                        psl = slice(2 * p, 2 * p + 2)
                        for sw in ("wqh", "wql"):
                            nc.tensor.matmul(
                                ps[:],
                                w_sb[sw][:, psl, msl],
                                xh_sb[:, psl, jsl],
                                start=(i == 0),
                                stop=(i == n_mm - 1),
                                perf_mode=DR,
                            )
                            i += 1
                    nc.vector.tensor_copy(q8_sb[:, m, jsl], ps[:])
                else:
                    n_mm = 3 * NP
                    i = 0
                    for sw, sx in (("wkh", xh_sb), ("wkl", xh_sb), ("wkh", xl_sb)):
                        for p in range(NP):
                            psl = slice(2 * p, 2 * p + 2)
                            nc.tensor.matmul(
                                ps[:],
                                w_sb[sw][:, psl, msl],
                                sx[:, psl, jsl],
                                start=(i == 0),
                                stop=(i == n_mm - 1),
                                perf_mode=DR,
                            )
                            i += 1
                    nc.vector.tensor_copy(k8_sb[:, m, 0, jsl], ps[:])
                    nc.vector.tensor_sub(
                        k8_sb[:, m, 1, jsl], ps[:], k8_sb[:, m, 0, jsl]
                    )

            def emit_v_group(st):
                ssl = slice(st * 128, st * 128 + 128)
                psv = gpool.tile([128, M], f32, tag="ps", name="psv")
                i = 0
                for sx, sw in ((xh_sb, "wvh"), (xh_sb, "wvl"), (xl_sb, "wvh")):
                    for p in range(NP):
                        psl = slice(2 * p, 2 * p + 2)
                        nc.tensor.matmul(
                            psv[:],
                            sx[:, psl, ssl],
                            w_sb[sw][:, psl, :],
                            start=(i == 0),
                            stop=(i == 3 * NP - 1),
                            perf_mode=DR,
                        )
                        i += 1
                vdst = v_sb[:, st].rearrange("p (h c) -> p h c", h=HL)[:, :, 0:DH]
                nc.vector.tensor_copy(vdst, psv[:].rearrange("p (h c) -> p h c", h=HL))

            def emit_outproj_group(st, n, copy_dve=False, dma_act=False):
                ssl = slice(st * 128, st * 128 + 128)
                nsl = slice(n * 512, (n + 1) * 512)
                ops = gpool.tile([128, 512], f32, tag="ps", name="ops")
                for p2 in range(MQ):
                    nc.tensor.matmul(
                        ops[:],
                        ctx_sb[:, p2, ssl],
                        wo_sb[:, p2, nsl],
                        start=(p2 == 0),
                        stop=(p2 == MQ - 1),
                    )
                o_sb = opool.tile([128, 512], f32, tag="o")
                if copy_dve:
                    nc.vector.tensor_copy(o_sb[:], ops[:])
                else:
                    nc.gpsimd.tensor_copy(o_sb[:], ops[:])
                if dma_act:
                    nc.scalar.dma_start(out[ssl, nsl], o_sb[:])
                else:
                    nc.sync.dma_start(out[ssl, nsl], o_sb[:])

            # fillers: PE work (next blocks' projections, previous blocks'
            # output projection) with deadlines, spread evenly over the
            # global attention pair-step sequence to fill PE stalls on ACT
            def run_attention_steps():
                def emit_scores_pair(j, h, skp):
                    """scores + exp + causal mask for one ski-pair; returns
                    what the (deferred) PV pair needs."""
                    hm, po = h // 2, 64 * (h % 2)
                    qrow = slice(po, po + DH)
                    sc = spool.tile([128, 2, W], f32, tag="sc")
                    ex = [
                        max(0, 128 * (skp + t) - j * W)
                        if skp + t >= TPB * j else 0
                        for t in range(2)
                    ]
                    for t in range(2):
                        ski = skp + t
                        # both slots cover the pair's union range so the
                        # paired exp reads fully-written psum; the extra
                        # columns in slot 1 are causally dead (PV skips)
                        q_mov = q8_sb[qrow, hm:hm + 1,
                                      j * W + ex[0]:(j + 1) * W]
                        nc.tensor.matmul(
                            sc[:, t, ex[0]:W],
                            k8_sb[qrow, hm, :, ski * 128:ski * 128 + 128],
                            q_mov.broadcast_to((DH, 2, W - ex[0])),
                            start=True,
                            stop=True,
                            perf_mode=DR,
                        )
                    attn = apool.tile([128, 2, W], bf16, tag="attn")
                    nc.scalar.activation(
                        attn[:, :, ex[0]:W], sc[:, :, ex[0]:W], A.Exp,
                        scale=scale,
                    )
                    for t in range(2):
                        ski = skp + t
                        if ski >= TPB * j:  # diagonal-crossing tile
                            cross_end = 128 * ski + 128 - j * W
                            nc.gpsimd.affine_select(
                                out=attn[:, t, ex[t]:cross_end],
                                in_=attn[:, t, ex[t]:cross_end],
                                compare_op=Alu.is_ge,
                                fill=0.0,
                                base=j * W + ex[t] - 128 * ski,
                                pattern=[[1, cross_end - ex[t]]],
                                channel_multiplier=-1,
                            )
                    return attn, ex

                def emit_pv_pair(ctx_ps, j, h, skp, attn, ex):
                    hv = slice(h * (DH + 1), (h + 1) * (DH + 1))
                    nski = TPB * (j + 1)
                    for t in range(2):
                        ski = skp + t
                        nc.tensor.matmul(
                            ctx_ps[:, ex[t]:W],
                            v_sb[:, ski, hv],
                            attn[:, t, ex[t]:W],
                            start=(ski == 0),
                            stop=(ski == nski - 1),
                        )

                def emit_norm(ctx_ps, j, h):
                    hm, po = h // 2, 64 * (h % 2)
                    dcp = rpool.tile([1, W], f32, tag="d")
                    rcp = rpool.tile([1, W], f32, tag="r")
                    bc = rpool.tile([64, W], f32, tag="bc")
                    nc.vector.tensor_copy(dcp[:], ctx_ps[DH:DH + 1, :])
                    nc.vector.reciprocal_approx_fast(out=rcp[:], in_=dcp[:])
                    nc.gpsimd.partition_broadcast(bc[:], rcp[:], channels=64)
                    nc.vector.tensor_mul(
                        ctx_sb[po:po + DH, hm, slice(j * W, (j + 1) * W)],
                        ctx_ps[0:DH, :], bc[:],
                    )

                # global step list over (block, head, ski-pair)
                steps = [(j, h, skp) for j in range(NJ) for h in range(HL)
                         for skp in range(0, TPB * (j + 1), 2)]
                sidx = {s: i for i, s in enumerate(steps)}
                n_steps = len(steps)

                # fillers with (earliest, deadline) step indices
                fillers = []
                for j in range(1, NJ):
                    e0 = sidx[(j - 1, 0, 0)]
                    for m in range(MQ):
                        dl = sidx[(j, 2 * m, 0)]
                        fillers.append([e0, dl, lambda j=j, m=m:
                                        emit_qk_group(j, m, "q")])
                        fillers.append([e0, dl, lambda j=j, m=m:
                                        emit_qk_group(j, m, "k")])
                    for st in range(j * TPB, (j + 1) * TPB):
                        dl = sidx[(j, 0, st - st % 2)]
                        fillers.append([e0, dl, lambda st=st:
                                        emit_v_group(st)])
                for j in range(NJ - 1):
                    e0 = sidx[(j + 1, 0, 0)]
                    for st in range(j * TPB, (j + 1) * TPB):
                        for n in range(D // 512):
                            # hold the last few groups back from the pacer:
                            # they fill PE during the final softmax chain
                            held = (j == NJ - 2 and st >= (j + 1) * TPB - 2)
                            fillers.append([10**9 if held else e0, n_steps,
                                            lambda st=st, n=n:
                                            emit_outproj_group(st, n)])
                fillers.sort(key=lambda f: f[1])
                acc = [0.0]

                def emit_forced(i):
                    for f in fillers:
                        if f[2] is not None and f[1] <= i:
                            fn, f[2] = f[2], None
                            fn()

                def pace(i):
                    left = sum(1 for f in fillers if f[2] is not None)
                    acc[0] += left / max(1, n_steps - i)
                    while acc[0] >= 1.0:
                        acc[0] -= 1.0
                        for f in fillers:
                            if f[2] is not None and f[0] <= i:
                                fn, f[2] = f[2], None
                                fn()
                                break
                        else:
                            break

                # software-pipelined: scores run one ski-pair ahead of PV so
                # the PV's wait on exp is hidden behind real PE work
                ctx_tiles = {}
                prev = None
                for i, (j, h, skp) in enumerate(steps):
                    emit_forced(i)
                    if skp == 0:
                        ctx_tiles[(j, h)] = cpool.tile(
                            [DH + 1, W], f32, tag="ctx", name="ctx_ps")
                    cur = (j, h, skp, emit_scores_pair(j, h, skp))
                    if prev is not None:
                        pj, ph, pskp, (pattn, pex) = prev
                        pace(i)
                        emit_pv_pair(ctx_tiles[(pj, ph)], pj, ph, pskp,
                                     pattn, pex)
                        if pskp + 2 >= TPB * (pj + 1):
                            emit_norm(ctx_tiles[(pj, ph)], pj, ph)
                    prev = cur
                pj, ph, pskp, (pattn, pex) = prev
                emit_pv_pair(ctx_tiles[(pj, ph)], pj, ph, pskp, pattn, pex)
                emit_norm(ctx_tiles[(pj, ph)], pj, ph)
                # drain leftover fillers
                for f in fillers:
                    if f[2] is not None:
                        fn, f[2] = f[2], None
                        fn()

            # ------------- main schedule --------------------------------
            # DMA transfers serialize on one engine in practice, so order
            # them by first use: q weights, x block 0, k weights, ...
            def wdma(n):
                nc.sync.dma_start(w_sb[n][:], w_r[n])

            wdma("wqh"), wdma("wql")
            emit_x_dmas(0, "h")
            wdma("wkh"), wdma("wkl")
            emit_x_dmas(0, "l")
            wdma("wvh"), wdma("wvl")
            if NJ > 1:
                emit_x_dmas(1)
            if NJ > 2:
                emit_x_dmas(2)
            nc.sync.dma_start(wo_sb[:], wo.rearrange("(q p) d -> p q d", p=128))
            for j in range(3, NJ):
                emit_x_dmas(j)
            for m in range(MQ):
                emit_qk_group(0, m, "q")
                emit_qk_group(0, m, "k")
            for st in range(TPB):
                emit_v_group(st)
            run_attention_steps()
            # last block's output projection: alternate copy/DMA engines so
            # the drain is not serialized on one queue
            i = 0
            for st in range((NJ - 1) * TPB, NJ * TPB):
                for n in range(D // 512):
                    emit_outproj_group(st, n, copy_dve=(i % 2 == 0),
                                       dma_act=(i % 2 == 1))
                    i += 1

    nc.compile()
    return nc


def _get_program():
    key = (S, D, HL, DH)
    if key not in _PROGRAM_CACHE:
        _PROGRAM_CACHE[key] = build_program(*key)
    return _PROGRAM_CACHE[key]


def _split8(a):
    """hi/lo e4m3 split of a float32 array."""
    hi = a.astype(E4)
    lo = (a - hi.astype(np.float32)).astype(E4)
    return hi, lo


def prep_core_inputs(xT, Wq, Wk, Wv, Wo, g):
    """Per-core input map.  xT: [D, S] f32 (one batch, transposed);
    W*: full [D, D] f32; g: head-group index (0..NCORES//B-1)."""
    sl = slice(HL * DH * g, HL * DH * (g + 1))
    xh8, xl8 = _split8(xT)
    m = {"xh": xh8, "xl": xl8}
    for name, Wfull in (("wq", Wq), ("wk", Wk), ("wv", Wv)):
        ws = np.ascontiguousarray(Wfull[sl, :].T) * WSC
        m[name + "h"], m[name + "l"] = _split8(ws)
    m["wo"] = (np.ascontiguousarray(Wo[:, sl].T) / WSC).astype(BF)
    return m


def make_in_maps(x, Wq, Wk, Wv, Wo):
    x = np.asarray(x, dtype=np.float32)
    Wq = np.asarray(Wq, dtype=np.float32)
    Wk = np.asarray(Wk, dtype=np.float32)
    Wv = np.asarray(Wv, dtype=np.float32)
    Wo = np.asarray(Wo, dtype=np.float32)
    # x split once per batch, shared across the 4 head-group cores
    xs = [_split8(np.ascontiguousarray(x[b].T)) for b in range(B)]
    in_maps = []
    for c in range(NCORES):
        b, g = divmod(c, NCORES // B)
        sl = slice(HL * DH * g, HL * DH * (g + 1))
        m = {"xh": xs[b][0], "xl": xs[b][1]}
        for name, Wfull in (("wq", Wq), ("wk", Wk), ("wv", Wv)):
            ws = np.ascontiguousarray(Wfull[sl, :].T) * WSC
            m[name + "h"], m[name + "l"] = _split8(ws)
        m["wo"] = (np.ascontiguousarray(Wo[:, sl].T) / WSC).astype(BF)
        in_maps.append(m)
    return in_maps


def kernel(x, Wq, Wk, Wv, Wo, bo):
    from concourse import bass2jax

    nc = _get_program()
    in_maps = make_in_maps(x, Wq, Wk, Wv, Wo)
    res = bass2jax.run_bass_via_pjrt(nc, in_maps, n_cores=NCORES)
    outs = [res[c]["out"] for c in range(NCORES)]
    gpb = NCORES // B
    o = np.stack([sum(outs[b * gpb + g] for g in range(gpb)) for b in range(B)])
    o = o + np.asarray(bo, dtype=np.float32)[None, None, :]
    return o.astype(np.float32)
